# revision 13
# baseline (speedup 1.0000x reference)
"""Trainium2 Bass kernel for nn_DEC_LargeCNN2Int (turbo-decoder CNN).

Data-parallel over 8 NeuronCores (32 samples each, 2 groups of 16 in an
octet layout: 8 partition rows per sample). Per core, per sample: 12
stack-instances of [conv0(7->100,K5) + 4x conv(100->100,K5), ELU] +
linear, with token interleaving between stacks done on-chip via GPSIMD
ap_gather.

v2: bf16 stationary weights (pipelined LDWEIGHTS), conv bias folded into
the matmul via a const-1 contraction row, activations stored as
z = ELU(y)+1 with the -1 shift folded into the next layer's weights
(halo cols hold 1.0), 3-op ELU (DVE min -> ACT exp -> fused STT),
on-device input prep from raw received, lin results DMA'd straight from
PSUM, and a cached PJRT runner so warm kernel() calls skip re-jit.
"""
import numpy as np

import concourse.bass as bass
import concourse.mybir as mybir
import concourse.tile as tile
from concourse import bacc

F32 = mybir.dt.float32
F32R = mybir.dt.float32r
BF16 = mybir.dt.bfloat16
I16 = mybir.dt.int16
AF = mybir.ActivationFunctionType
ALU = mybir.AluOpType

B, L, FT, NUM_ITER, NL, UNIT, K = 256, 2048, 5, 6, 5, 100, 5
N_CORES = 8
TAPS = [2, 0, 1, 3, 4]
NCH = L // 512                  # 4 psum chunks
CR = UNIT                       # const-1 row index (bias row)
NPR = UNIT + 1                  # conv contraction rows (100 ch + const)
NLIN = UNIT + 1 + FT            # lin contraction rows (+5 prior rows)
PB = 128                        # weight block column pitch (FWL needs 128)


def _wrap_idx(t, groups):
    """ap_gather index layout: idx j at [j%16, j//16], replicated per group."""
    w = np.zeros((16, L // 16), np.int16)
    w[np.arange(L) % 16, np.arange(L) // 16] = t.astype(np.int16)
    return np.tile(w, (groups, 1))


def build_host_inputs(inputs, n_iter=NUM_ITER):
    """Host-side prep. Returns per-core list of input dicts."""
    n_inst = 2 * n_iter
    bpc = B // N_CORES

    p1 = np.asarray(inputs['p_array1']).astype(np.int64)
    p2 = np.asarray(inputs['p_array2']).astype(np.int64)
    inv1 = np.argsort(p1)
    inv2 = np.argsort(p2)
    t1 = inv2[p1]
    t2 = inv1[p2]

    received = np.asarray(inputs['received'], np.float32)
    recv_t = received.transpose(2, 0, 1)    # [3, B, L] view

    conv0_w = np.asarray(inputs['conv0_w'], np.float32)
    conv0_b = np.asarray(inputs['conv0_b'], np.float32)
    convs_w = np.asarray(inputs['convs_w'], np.float32)
    convs_b = np.asarray(inputs['convs_b'], np.float32)
    lin1_w = np.asarray(inputs['lin1_w'], np.float32)
    lin1_b = np.asarray(inputs['lin1_b'], np.float32)
    lin2_w = np.asarray(inputs['lin2_w'], np.float32)
    lin2_b = np.asarray(inputs['lin2_b'], np.float32)
    lin2_last_w = np.asarray(inputs['lin2_last_w'], np.float32)
    lin2_last_b = np.asarray(inputs['lin2_last_b'], np.float32)

    # conv0 lhsT block for one 32-row j-group: [32, 20*PB] (j-groups identical;
    # blocks padded to 128 cols so bf16 fast-weight-load triggers)
    c0m = np.zeros((n_inst, 32, 20 * PB), np.float32)
    # mid-layer lhsT: [101, 20*PB]; row 100 = bias - sum(W) (z-shift fold)
    cwm = np.zeros((n_inst, NPR, 20 * PB), np.float32)
    # lin lhsT: rows 0..99 w^T, row 100 bias', rows 101..105 -I (extrinsic)
    linw = np.zeros((NLIN, n_inst, FT), np.float32)
    linlast = np.zeros((NLIN, 1), np.float32)
    linlast[:UNIT, 0] = lin2_last_w[0]
    linlast[CR, 0] = lin2_last_b[0] - lin2_last_w[0].sum()

    for inst in range(n_inst):
        idx, col = divmod(inst, 2)
        w0 = conv0_w[idx, col]              # [100, 7, 5]
        b0 = conv0_b[idx, col]              # [100]
        for v in range(4):
            for k in range(K):
                blk = c0m[inst, :, (v * K + k) * PB:(v * K + k) * PB + UNIT]
                blk[8 * v:8 * v + 7, :] = w0[:, :, k].T
                if k == 2:
                    blk[8 * v + 7, :] = b0
        for li in range(1, NL):
            wl = convs_w[idx, col, li - 1]  # [100out, 100in, 5]
            bl = convs_b[idx, col, li - 1]
            for k in range(K):
                cwm[inst, :UNIT, ((li - 1) * K + k) * PB:
                    ((li - 1) * K + k) * PB + UNIT] = wl[:, :, k].T
            cwm[inst, CR, ((li - 1) * K + 2) * PB:
                ((li - 1) * K + 2) * PB + UNIT] = bl - wl.sum(axis=(1, 2))
        if inst < n_inst - 1:
            lw = lin1_w[idx] if col == 0 else lin2_w[idx]
            lb = lin1_b[idx] if col == 0 else lin2_b[idx]
            linw[:UNIT, inst, :] = lw.T
            linw[CR, inst, :] = lb - lw.sum(axis=1)
            if inst > 0:
                linw[CR + 1:CR + 1 + FT, inst, :] = -np.eye(FT, dtype=np.float32)

    import ml_dtypes

    shared = {
        'c0m': c0m.astype(ml_dtypes.bfloat16),
        'cwm': cwm.astype(ml_dtypes.bfloat16),
        'linw': np.ascontiguousarray(linw.reshape(NLIN, -1)).astype(
            ml_dtypes.bfloat16),
        'linlast': linlast.astype(ml_dtypes.bfloat16),
        'it1': _wrap_idx(t1, 8),
        'it2': _wrap_idx(t2, 8),
        'io': _wrap_idx(inv2, 1),
        'ipm': np.concatenate(
            [_wrap_idx(q, 1) for q in (p1, t1, p2, t2)], axis=1),
    }

    per_core = []
    for c in range(N_CORES):
        lo = c * bpc
        m = dict(shared)
        m['recv'] = np.ascontiguousarray(recv_t[:, lo:lo + bpc, :])
        per_core.append(m)
    return per_core


def build_program(n_iter=NUM_ITER):
    n_inst = 2 * n_iter
    n_groups = 2
    spg = 16

    nc = bacc.Bacc('TRN2', target_bir_lowering=False, debug=False)

    recv_d = nc.dram_tensor("recv", [3, 32, L], F32, kind="ExternalInput")
    c0m_d = nc.dram_tensor("c0m", [n_inst, 32, 20 * PB], BF16,
                           kind="ExternalInput")
    cwm_d = nc.dram_tensor("cwm", [n_inst, NPR, 20 * PB], BF16,
                           kind="ExternalInput")
    lw_d = nc.dram_tensor("linw", [NLIN, n_inst * FT], BF16,
                          kind="ExternalInput")
    ll_d = nc.dram_tensor("linlast", [NLIN, 1], BF16, kind="ExternalInput")
    it1_d = nc.dram_tensor("it1", [128, L // 16], I16, kind="ExternalInput")
    it2_d = nc.dram_tensor("it2", [128, L // 16], I16, kind="ExternalInput")
    io_d = nc.dram_tensor("io", [16, L // 16], I16, kind="ExternalInput")
    ipm_d = nc.dram_tensor("ipm", [16, 4 * (L // 16)], I16,
                           kind="ExternalInput")
    out_d = nc.dram_tensor("out", [n_groups * spg, L], F32,
                           kind="ExternalOutput")

    with tile.TileContext(nc) as tc:
        with tc.tile_pool(name="persist", bufs=1) as pp, \
             tc.tile_pool(name="wts", bufs=2) as wp, \
             tc.tile_pool(name="elu", bufs=4) as ep, \
             tc.tile_pool(name="staging", bufs=1) as ip, \
             tc.tile_pool(name="ps", bufs=2, space="PSUM") as ps:

            TA = [pp.tile([128, L + 4], BF16, tag=f"TA{g}", name=f"TA{g}") for g in range(n_groups)]
            TB = [pp.tile([128, L + 4], BF16, tag=f"TB{g}", name=f"TB{g}") for g in range(n_groups)]
            Tf = [pp.tile([128, L], F32, tag=f"Tf{g}", name=f"Tf{g}") for g in range(n_groups)]
            S1 = [pp.tile([128, L], F32, tag=f"S1{g}", name=f"S1{g}") for g in range(n_groups)]
            S2 = [pp.tile([128, L], F32, tag=f"S2{g}", name=f"S2{g}") for g in range(n_groups)]
            XB = [[pp.tile([NLIN if i == 1 else NPR, L + 4], BF16,
                           tag=f"XB{p}_{i}", name=f"XB{p}_{i}") for i in range(3)]
                  for p in range(2)]
            linw_t = pp.tile([NLIN, n_inst * FT], BF16, tag="linw")
            linlast_t = pp.tile([NLIN, 1], BF16, tag="linlast")
            it1_t = pp.tile([128, L // 16], I16, tag="it1")
            it2_t = pp.tile([128, L // 16], I16, tag="it2")
            io_t = pp.tile([16, L // 16], I16, tag="io")
            ipm_t = pp.tile([16, 4 * (L // 16)], I16, tag="ipm")

            # ---- init ----
            nc.sync.dma_start(out=linw_t, in_=lw_d[:, :])
            nc.sync.dma_start(out=linlast_t, in_=ll_d[:, :])
            nc.sync.dma_start(out=it1_t, in_=it1_d[:, :])
            nc.sync.dma_start(out=it2_t, in_=it2_d[:, :])
            nc.sync.dma_start(out=io_t, in_=io_d[:, :])
            nc.sync.dma_start(out=ipm_t, in_=ipm_d[:, :])

            for g in range(n_groups):
                nc.vector.memset(TA[g][:, :], 0.0)
                nc.vector.memset(TB[g][:, :], 0.0)
                nc.vector.memset(S1[g][:, :], 0.0)
                nc.vector.memset(S2[g][:, :], 0.0)
            ones = ip.tile([16, L], F32, tag="R", name="ones")
            nc.vector.memset(ones[:, :], 1.0)
            for g in range(n_groups):
                nc.sync.dma_start(out=S1[g][7:128:8, :], in_=ones[:, :])
                nc.sync.dma_start(out=S2[g][7:128:8, :], in_=ones[:, :])
            for pset in XB:
                for i, xb in enumerate(pset):
                    nc.vector.memset(xb[:, :], 0.0)
                    # z-type halo (x=0 -> z=1) + const-1 bias row
                    nc.vector.memset(xb[0:NPR, 0:2], 1.0)
                    nc.vector.memset(xb[0:NPR, L + 2:L + 4], 1.0)
                    # const-1 bias row (rows 96..99 are data rows: their halo
                    # stays 1.0 which is the correct z-form padding, and the
                    # first ELU overwrites their payload)
                    nc.vector.memset(xb[96:NPR, :], 1.0)

            # raw received components -> octet state tiles via gathers
            # (r, dest tile, dest octet row, ipm idx block)
            specs = [(0, 0, 0, 0), (2, 0, 1, 1), (0, 1, 0, 2), (1, 1, 1, 3)]
            for g in range(n_groups):
                for r, sd, row, b in specs:
                    R = ip.tile([16, L], F32, tag="R", name="R")
                    G = ip.tile([16, L], F32, tag="G", name="G")
                    Sd = S1[g] if sd == 0 else S2[g]
                    nc.sync.dma_start(out=R, in_=recv_d[r, g * 16:(g + 1) * 16, :])
                    nc.gpsimd.ap_gather(G[:, :], R[:, :],
                                        ipm_t[0:16, b * 128:(b + 1) * 128],
                                        channels=16, num_elems=L, d=1,
                                        num_idxs=L)
                    nc.sync.dma_start(out=Sd[row:128:8, :], in_=G[:, :])

            def elu(psum, xout):
                """xout[0:100, 2:L+2] = 1 + elu(psum[0:100, :])  (z-form)."""
                m_t = ep.tile([UNIT, L], BF16, tag="m", name="m_t")
                e_t = ep.tile([UNIT, L], BF16, tag="e", name="e_t")
                nc.vector.tensor_scalar_min(m_t, psum[0:UNIT, :], 0.0)
                nc.scalar.activation(e_t, m_t, AF.Exp)
                nc.vector.scalar_tensor_tensor(
                    xout[0:UNIT, 2:L + 2], psum[0:UNIT, :], 0.0,
                    e_t, ALU.max, ALU.add)

            for inst in range(n_inst):
                idx, col = divmod(inst, 2)
                last = inst == n_inst - 1
                c0t = wp.tile([128, 20 * PB], BF16, tag="c0t")
                cwt = wp.tile([NPR, 20 * PB], BF16, tag="cwt")
                for j in range(4):
                    nc.sync.dma_start(out=c0t[32 * j:32 * j + 32, :],
                                      in_=c0m_d[inst])
                nc.sync.dma_start(out=cwt, in_=cwm_d[inst])

                # interleaver gathers straight into the halo'd conv0 input
                for g in range(n_groups):
                    if col == 0:
                        nc.gpsimd.ap_gather(Tf[g][:, :], S2[g][:, :],
                                            it1_t[:, :], channels=128,
                                            num_elems=L, d=1, num_idxs=L)
                        nc.scalar.copy(TA[g][:, 2:L + 2], Tf[g][:, :])
                    else:
                        nc.gpsimd.ap_gather(Tf[g][:, :], S1[g][:, :],
                                            it2_t[:, :], channels=128,
                                            num_elems=L, d=1, num_idxs=L)
                        nc.scalar.copy(TB[g][:, 2:L + 2], Tf[g][:, :])

                for g in range(n_groups):
                    T = TA[g] if col == 0 else TB[g]

                    def conv_layer(sp, si, li):
                        j, v = divmod(si, 4)
                        pt = ps.tile([128, L], F32, tag="ps", name="pt")
                        for t, k in enumerate(TAPS):
                            for c in range(NCH):
                                if li == 0:
                                    nc.tensor.matmul(
                                        pt[0:32, c * 512:(c + 1) * 512]
                                        if False else
                                        pt[:, c * 512:(c + 1) * 512],
                                        c0t[32 * j:32 * j + 32,
                                            (v * K + k) * PB:
                                            (v * K + k) * PB + 128],
                                        T[32 * j:32 * j + 32,
                                          c * 512 + k:c * 512 + k + 512],
                                        start=(t == 0), stop=(t == 4),
                                        tile_position=(32 * j, 0))
                                else:
                                    xin = XB[sp][(li - 1) % 3]
                                    nc.tensor.matmul(
                                        pt[:, c * 512:(c + 1) * 512],
                                        cwt[:, ((li - 1) * K + k) * PB:
                                            ((li - 1) * K + k) * PB + 128],
                                        xin[0:NPR,
                                            c * 512 + k:c * 512 + k + 512],
                                        start=(t == 0), stop=(t == 4))
                        elu(pt, XB[sp][li % 3])

                    def lin_stage(sp, si):
                        x5 = XB[sp][1]
                        if inst > 0:
                            nc.sync.dma_start(
                                out=x5[CR + 1:CR + 1 + FT, 2:L + 2],
                                in_=T[8 * si + 2:8 * si + 7, 2:L + 2])
                        m = 1 if last else FT
                        wsl = linlast_t[:, 0:1] if last \
                            else linw_t[:, inst * FT:inst * FT + FT]
                        pslin = ps.tile([m, L], F32, tag="ps", name="pslin")
                        for c in range(NCH):
                            nc.tensor.matmul(
                                pslin[:, c * 512:(c + 1) * 512], wsl,
                                x5[0:NLIN, c * 512 + 2:c * 512 + 514],
                                start=True, stop=True)
                        if last:
                            stg1 = ip.tile([1, L], F32, tag="stg1", name="stg1")
                            nc.scalar.activation(stg1, pslin[0:1, :], AF.Tanh,
                                                 scale=0.5)
                            nc.vector.tensor_scalar(stg1, stg1, 0.5, 0.5,
                                                    ALU.mult, ALU.add)
                            nc.sync.dma_start(out=Tf[g][si:si + 1, :],
                                              in_=stg1)
                        else:
                            Sd = S1[g] if col == 0 else S2[g]
                            stg = ip.tile([FT, L], F32, tag="stg", name="stg")
                            nc.scalar.copy(stg, pslin[0:FT, :])
                            nc.sync.dma_start(
                                out=Sd[8 * si + 2:8 * si + 7, :],
                                in_=stg)

                    for p in range(8):
                        sA = (p // 4) * 8 + p % 4      # j-group 0/2
                        sB = sA + 4                    # j-group 1/3
                        for li in range(NL):
                            conv_layer(0, sA, li)
                            conv_layer(1, sB, li)
                        lin_stage(0, sA)
                        lin_stage(1, sB)

            # final: out[l] = sig[inv2[l]]
            for g in range(n_groups):
                nc.gpsimd.ap_gather(S1[g][0:16, :], Tf[g][0:16, :],
                                    io_t[0:16, :], channels=16,
                                    num_elems=L, d=1, num_idxs=L)
                nc.sync.dma_start(out=out_d[g * spg:g * spg + spg, :],
                                  in_=S1[g][0:spg, :])

    nc.compile()
    return nc


_PROG_CACHE = {}
_RUNNER_CACHE = {}


def _get_prog(n_iter=NUM_ITER):
    if n_iter not in _PROG_CACHE:
        _PROG_CACHE[n_iter] = build_program(n_iter)
    return _PROG_CACHE[n_iter]


class _Runner:
    """Persistent jitted PJRT executor (avoids per-call re-jit/re-compile)."""

    def __init__(self, nc, n_cores):
        import jax
        from jax.experimental.shard_map import shard_map
        from jax.sharding import Mesh, NamedSharding, PartitionSpec

        import concourse.bass2jax as b2j

        b2j.install_neuronx_cc_hook()
        self.nc = nc
        self.n = n_cores
        partition_name = (
            nc.partition_id_tensor.name if nc.partition_id_tensor else None
        )
        in_names, out_names, out_avals, zero_outs = [], [], [], []
        for alloc in nc.m.functions[0].allocations:
            if not isinstance(alloc, mybir.MemoryLocationSet):
                continue
            name = alloc.memorylocations[0].name
            if alloc.kind == "ExternalInput":
                if name != partition_name:
                    in_names.append(name)
            elif alloc.kind == "ExternalOutput":
                shape = tuple(alloc.tensor_shape)
                dtype = mybir.dt.np(alloc.dtype)
                out_names.append(name)
                out_avals.append(jax.core.ShapedArray(shape, dtype))
                zero_outs.append(np.zeros(shape, dtype))
        self.in_names = list(in_names)
        self.out_names = out_names
        self.out_avals = out_avals
        n_params = len(in_names)
        n_outs = len(out_names)
        all_names = in_names + out_names
        if partition_name is not None:
            all_names.append(partition_name)
        donate = tuple(range(n_params, n_params + n_outs))

        def _body(*args):
            operands = list(args)
            if partition_name is not None:
                operands.append(b2j.partition_id_tensor())
            outs = b2j._bass_exec_p.bind(
                *operands,
                out_avals=tuple(out_avals),
                in_names=tuple(all_names),
                out_names=tuple(out_names),
                lowering_input_output_aliases=(),
                sim_require_finite=True,
                sim_require_nnan=True,
                nc=nc,
            )
            return tuple(outs)

        devices = jax.devices()[:n_cores]
        mesh = Mesh(np.asarray(devices), ("core",))
        self.sharding = NamedSharding(mesh, PartitionSpec("core"))
        in_specs = (PartitionSpec("core"),) * (n_params + n_outs)
        out_specs = (PartitionSpec("core"),) * n_outs
        self.fn = jax.jit(
            shard_map(_body, mesh=mesh, in_specs=in_specs,
                      out_specs=out_specs, check_rep=False),
            donate_argnums=donate,
            keep_unused=True,
        )
        self.concat_zero = [
            np.zeros((n_cores * z.shape[0], *z.shape[1:]), z.dtype)
            for z in zero_outs
        ]
        self._jax = jax

    def run(self, per_core):
        jax = self._jax
        n = self.n
        concat_in = [
            np.concatenate([np.asarray(per_core[c][name]) for c in range(n)],
                           axis=0)
            for name in self.in_names
        ]
        dev_in = [jax.device_put(a, self.sharding) for a in concat_in]
        zeros = [jax.device_put(z, self.sharding) for z in self.concat_zero]
        outs = self.fn(*dev_in, *zeros)
        return [
            {
                name: np.asarray(outs[i]).reshape(n, *self.out_avals[i].shape)[c]
                for i, name in enumerate(self.out_names)
            }
            for c in range(n)
        ]


def _get_runner(nc, cores):
    key = (id(nc), cores)
    if key not in _RUNNER_CACHE:
        _RUNNER_CACHE[key] = _Runner(nc, cores)
    return _RUNNER_CACHE[key]


def run(inputs, n_iter=NUM_ITER, cores=N_CORES):
    nc = _get_prog(n_iter)
    per_core = build_host_inputs(inputs, n_iter)
    runner = _get_runner(nc, cores)
    return runner.run(per_core[:cores])


def kernel(**inputs):
    res = run(inputs)
    bpc = B // N_CORES
    out = np.empty((B, L, 1), np.float32)
    for c in range(N_CORES):
        out[c * bpc:(c + 1) * bpc, :, 0] = res[c]["out"]
    return out
